# revision 42
# baseline (speedup 1.0000x reference)
"""Trainium2 Bass kernel for nn_Attention_54279796686851.

Multi-head attention over 4x768x8x14x14 spatial tokens + a cls token,
followed by an output projection.

Sharding (8 cores): core = (batch b, half hf). Each core computes 6 of the
12 heads of one batch (tensor parallel on heads), including the softmax
attention and a PARTIAL output projection over its 384 input channels
(row-sharded W). The host sums the two partial projections per batch and
adds the bias.

Device algorithm per core (all matmul operands bf16, fp32 PSUM accum):
  scoresT[k, q] = kT.T @ qT         (k-orientation: softmax dim on partitions)
  expS = exp(SCALE * scoresT)       (no max subtraction: logits ~ N(0,1))
  pv[0:64, q]  = V.T @ expS         (V has a fused ones-column ->
  pv[64, q]    = sum_k expS)         denominator comes out of the matmul)
  xT[c, q] = pv[0:64] * (1/pv[64])  (recip broadcast across partitions via DMA)
  yT_partial = W_slice.T @ xT       (projection over this core's channels)
"""

import numpy as np
import ml_dtypes

import concourse.bass as bass
import concourse.tile as tile
from concourse import mybir
from concourse.bass_utils import run_bass_kernel_spmd

# ---------------------------------------------------------------------------
# Patch: this container's walrus rejects >1 sync wait on CTRL instructions
# (Drain/NoOp). Distribute the Tile postamble drain's sem waits one-per-nop.
# ---------------------------------------------------------------------------
from bass_rust import ScopedClock


def _patched_drain_and_barrier(self, tick_clock, wait_clock):
    collector = self.nc.sync.nop(nofuse=True)
    wait_clock.add_sem_waits(
        collector.ins, ScopedClock({None: tick_clock.global_clock})
    )
    si = collector.ins.sync_info
    waits = list(si.on_wait) if si is not None else []
    if len(waits) > 1:
        si.on_wait = waits[:1]
        for w in waits[1:]:
            n = self.nc.sync.nop(nofuse=True)
            nsi = n.ins.sync_info
            if nsi is None:
                n.ins.sync_info = type(si)(on_wait=[w], on_update=[])
            else:
                nsi.on_wait = [w]
    self.nc.sync.drain()
    self.nc.all_engine_barrier()
    assert self.sems is not None
    popped = self.nc._tile_sem_poison_stack.pop()
    assert popped is self._sem_poison
    self.nc.clear_and_free_semaphores(list(self.sems.allocated().values()))
    self.nc.all_engine_barrier()


tile.TileContext._drain_and_barrier = _patched_drain_and_barrier

_wsplit_counter = [0]


def _split_sync_waits(nc):
    """Walrus in this container allows at most one sync wait per instruction
    (and none on Drain). Spill extra waits onto same-engine NoOps inserted
    just before the instruction — strictly more conservative, so safe."""
    import bass_rust as _br

    n_split = 0
    for fn in nc.m.functions:
        for bb in fn.blocks:
            insts = list(bb.instructions)
            out = []
            changed = False
            for ins in insts:
                si = ins.sync_info
                waits = list(si.on_wait) if si is not None else []
                keep = 0 if type(ins).__name__ == "InstDrain" else 1
                if len(waits) > keep:
                    n_extra = len(waits) - keep
                    spill, remain = waits[:n_extra], waits[n_extra:]
                    si.on_wait = remain
                    for w in spill:
                        _wsplit_counter[0] += 1
                        nop = mybir.InstNoOp(
                            name=f"wsplit_{_wsplit_counter[0]}",
                            ins=[], outs=[], engine=ins.engine,
                        )
                        nop.sync_info = _br.SyncInfo(on_wait=[w], on_update=[])
                        out.append(nop)
                        n_split += 1
                    changed = True
                out.append(ins)
            if changed:
                bb.instructions = out
    return n_split

# ---------------------------------------------------------------------------
# Problem constants (hardcoded per contract)
# ---------------------------------------------------------------------------
B, C, L, H, W_SP = 4, 768, 8, 14, 14
HEADS, HD = 12, 64
SCALE = HD ** (-0.5)
N_SP = L * H * W_SP          # 1568 spatial tokens
S = N_SP + 1                 # 1569 tokens incl cls
KT = 13                      # k tiles of 128
SK = KT * 128                # 1664 padded k tokens
HP = 3                       # head pairs per core
N_CORES = 8

BF16 = mybir.dt.bfloat16
F32 = mybir.dt.float32
EXP = mybir.ActivationFunctionType.Exp

# q processed in two uneven segments (the smaller one last to shrink the
# kernel tail); chunk boundaries stay inside one PSUM bank
QH = [(0, 1024), (1024, 545)]
SCW = 1024  # scores tile width
# (qh index, start within qh, width, absolute start)
PV_CHUNKS = [(0, 0, 512, 0), (0, 512, 512, 512), (1, 0, 512, 1024), (1, 512, 33, 1536)]
# projection chunks grouped by the q segment that produces them
PROJ_CHUNKS_QH = [[(0, 512), (512, 512)], [(1024, 512), (1536, 33)]]

_bf16 = ml_dtypes.bfloat16


def build_program(n_iters: int = 1):
    """Build the per-core Bass program. n_iters>1 repeats the body for
    wall-clock timing (same I/O each iteration)."""
    nc = bass.Bass()
    q_in = nc.dram_tensor("q_in", [HP, 128, S], BF16, kind="ExternalInput")
    k_in = nc.dram_tensor("k_in", [HP, 128, SK], BF16, kind="ExternalInput")
    vb_in = nc.dram_tensor("vb_in", [2 * HP, 128, KT * 65], BF16, kind="ExternalInput")
    wt_in = nc.dram_tensor("wt_in", [HP, 128, C], BF16, kind="ExternalInput")
    y_out = nc.dram_tensor("y_out", [6, 128, S], F32, kind="ExternalOutput")

    with tile.TileContext(nc) as tc:
        with (
            tc.tile_pool(name="const", bufs=1) as const,
            tc.tile_pool(name="io", bufs=1) as io,
            tc.tile_pool(name="exps", bufs=56) as exps,
            tc.tile_pool(name="prhs", bufs=1) as prhs,
            tc.tile_pool(name="small", bufs=2) as small,
            tc.tile_pool(name="outp", bufs=3) as outp,
            tc.tile_pool(name="scores", bufs=2, space="PSUM") as scores,
            tc.tile_pool(name="pvp", bufs=4, space="PSUM") as pvp,
            tc.tile_pool(name="drb", bufs=2, space="DRAM") as drb,
        ):
            wt_sb = const.tile([128, HP * C], BF16)
            ones_f = const.tile([1, 64], F32)
            nc.vector.memset(ones_f, 1.0)
            # prime the exp table set while input DMAs run (the first
            # activation otherwise pays the ~2.7us ACT_TABLE_LOAD inline)
            warm = const.tile([1, 64], F32)
            nc.scalar.activation(warm, ones_f, EXP, scale=1.0)
            ones_r = const.tile([1, 64], mybir.dt.float32r)
            nc.vector.tensor_copy(ones_r, ones_f)

            proj_rhs = []
            for k3 in range(HP):
                t = prhs.tile([128, S], BF16, name=f"proj_rhs{k3}", tag=f"proj_rhs{k3}")
                proj_rhs.append(t)

            def body():
                # load all three head-pairs' inputs upfront (two DMA paths);
                # pair 0's q/k are split so the first QK can start early
                qts, kts, vbs = [], [], []
                for hp in range(HP):
                    qt = io.tile([128, S], BF16, name=f"qt{hp}", tag=f"qt{hp}")
                    kt_sb = io.tile([128, SK], BF16, name=f"kt{hp}", tag=f"kt{hp}")
                    if hp == 0:
                        nc.sync.dma_start(kt_sb[:, 0:256], k_in[hp][:, 0:256])
                        nc.sync.dma_start(qt[:, 0:512], q_in[hp][:, 0:512])
                        nc.sync.dma_start(qt[:, 512:1024], q_in[hp][:, 512:1024])
                        nc.sync.dma_start(kt_sb[:, 256:SK], k_in[hp][:, 256:SK])
                        nc.sync.dma_start(qt[:, 1024:S], q_in[hp][:, 1024:S])
                    else:
                        nc.sync.dma_start(qt, q_in[hp])
                        nc.sync.dma_start(kt_sb, k_in[hp])
                    qts.append(qt)
                    kts.append(kt_sb)
                for hp in range(HP):
                    vba = io.tile([128, KT * 65], BF16, name=f"vba{hp}", tag=f"vba{hp}")
                    vbb = io.tile([128, KT * 65], BF16, name=f"vbb{hp}", tag=f"vbb{hp}")
                    nc.gpsimd.dma_start(vba, vb_in[2 * hp])
                    nc.gpsimd.dma_start(vbb, vb_in[2 * hp + 1])
                    vbs.append((vba, vbb))
                for k3 in range(HP):
                    nc.gpsimd.dma_start(wt_sb[:, k3 * C:(k3 + 1) * C], wt_in[k3])

                def norm_chunks(qh, hp, chunk_pvs, pe_bcast=False):
                    """recip + one 2-row broadcast + normalize muls for a
                    list of (chunk, pv_hd0, pv_hd1)."""
                    ctot = sum(c[0][2] for c in chunk_pvs)
                    rc = small.tile([65, SCW], F32, name="rc", tag="rc")
                    off = 0
                    for (ck, pv0, pv1) in chunk_pvs:
                        cw = ck[2]
                        nc.vector.reciprocal(rc[0:1, off:off + cw], pv0[64:65, :cw])
                        nc.vector.reciprocal(rc[64:65, off:off + cw], pv1[64:65, :cw])
                        off += cw
                    bc = small.tile([128, SCW], F32, name="bc", tag="bc")
                    if pe_bcast:
                        # tail path: broadcast on the PE via a K=1 float32r
                        # matmul (ACT copies psum->sbuf; both engines are
                        # otherwise idle in the tail)
                        assert ctot <= 512
                        rcra = small.tile([1, SCW], mybir.dt.float32r,
                                          name="rcra", tag="rcra")
                        rcrb = small.tile([1, SCW], mybir.dt.float32r,
                                          name="rcrb", tag="rcrb")
                        nc.vector.tensor_copy(rcra[0:1, 0:ctot], rc[0:1, 0:ctot])
                        nc.vector.tensor_copy(rcrb[0:1, 0:ctot], rc[64:65, 0:ctot])
                        bcp = pvp.tile([65, 512], F32, name="bcp", tag="pv")
                        bcq = pvp.tile([65, 512], F32, name="bcq", tag="pv")
                        cpad = min(512, (ctot + 63) // 64 * 64)
                        nc.tensor.matmul(bcp[0:64, 0:cpad], lhsT=ones_r,
                                         rhs=rcra[0:1, 0:cpad], start=True, stop=True)
                        nc.tensor.matmul(bcq[0:64, 0:cpad], lhsT=ones_r,
                                         rhs=rcrb[0:1, 0:cpad], start=True, stop=True)
                        nc.scalar.copy(bc[0:64, 0:ctot], bcp[0:64, 0:ctot])
                        nc.scalar.copy(bc[64:128, 0:ctot], bcq[0:64, 0:ctot])
                    else:
                        # broadcast both heads' recips across their head's 64
                        # partitions via a DRAM bounce (SBUF sources cannot
                        # have a 0-step partition AP): bc rows 0-63 <- rc row
                        # 0, rows 64-127 <- rc row 64
                        rb = drb.tile([2, SCW], F32, name="rb", tag="rb")
                        nc.gpsimd.dma_start(rb[0:1, 0:ctot], rc[0:1, 0:ctot])
                        nc.gpsimd.dma_start(rb[1:2, 0:ctot], rc[64:65, 0:ctot])
                        bsrc = bass.AP(tensor=rb.tensor, offset=rb.offset,
                                       ap=[[SCW, 2], [0, 64], [1, ctot]])
                        nc.gpsimd.dma_start(bc[:, 0:ctot], bsrc)
                    off = 0
                    for (ck, pv0, pv1) in chunk_pvs:
                        _, c0, cw, qabs = ck
                        for hd, pv in ((0, pv0), (1, pv1)):
                            nc.vector.tensor_mul(
                                proj_rhs[hp][hd * 64:hd * 64 + 64, qabs:qabs + cw],
                                pv[0:64, :cw],
                                bc[hd * 64:hd * 64 + 64, off:off + cw],
                            )
                        off += cw

                def proj_unit(mo, qc0, qcw, use_act=False):
                    yps = scores.tile([128, SCW], F32, name="yps", tag="sc")
                    s0 = 0
                    while s0 < qcw:  # per-PSUM-bank matmul sub-chunks
                        sw = min(512, qcw - s0)
                        for k3 in range(HP):
                            nc.tensor.matmul(
                                yps[:, s0:s0 + sw],
                                lhsT=wt_sb[:, k3 * C + mo * 128:k3 * C + (mo + 1) * 128],
                                rhs=proj_rhs[k3][:, qc0 + s0:qc0 + s0 + sw],
                                start=(k3 == 0), stop=(k3 == HP - 1),
                            )
                        s0 += sw
                    ob = outp.tile([128, SCW], F32, name="ob", tag="ob")
                    if use_act:
                        nc.scalar.copy(ob[:, :qcw], yps[:, :qcw])
                    else:
                        nc.vector.tensor_copy(ob[:, :qcw], yps[:, :qcw])
                    nc.sync.dma_start(y_out[mo][:, qc0:qc0 + qcw], ob[:, :qcw])

                def fused_phase(cur, prev, weave=(), fuse_own_pv=False):
                    """Emit QK+exp of phase `cur` interleaved (per k tile)
                    with the PV accumulation of phase `prev` and any extra
                    `weave` callables (projection units). With fuse_own_pv,
                    `cur`'s own PV chains are also emitted in-loop right
                    after each exp (used for the final phase)."""
                    weave = list(weave)
                    own_pvs = None
                    if cur is not None:
                        cqh, chp = cur
                        q0, qw = QH[cqh]
                        qt, kt_sb = qts[chp], kts[chp]
                        exp_tiles = [[None] * KT, [None] * KT]
                        if fuse_own_pv:
                            cchunks = [c for c in PV_CHUNKS if c[0] == cqh]
                            own_pvs = [
                                [pvp.tile([65, 512], F32, name="opv", tag="pv")
                                 for _ in cchunks]
                                for _ in range(2)
                            ]
                    else:
                        exp_tiles = None
                    if prev is not None:
                        pqh, php, pets = prev
                        pchunks = [c for c in PV_CHUNKS if c[0] == pqh]
                        vb = vbs[php]
                        pvs_by_hd = [
                            [pvp.tile([65, 512], F32, name="pv", tag="pv")
                             for _ in pchunks]
                            for _ in range(2)
                        ]
                    for k in range(KT):
                        if cur is not None:
                            scs = [scores.tile([128, SCW], F32, name="sc", tag="sc")
                                   for _ in range(2)]
                            c0 = 0
                            while c0 < qw:
                                cw = min(512, qw - c0)
                                # adjacent A/B matmuls live in different PE
                                # row groups (base partition 0 vs 64) and can
                                # execute concurrently in the array
                                for hd in range(2):
                                    r0, r1 = hd * 64, hd * 64 + 64
                                    nc.tensor.matmul(
                                        scs[hd][:, c0:c0 + cw],
                                        lhsT=kt_sb[r0:r1, k * 128:(k + 1) * 128],
                                        rhs=qt[r0:r1, q0 + c0:q0 + c0 + cw],
                                        start=True, stop=True,
                                    )
                                c0 += cw
                            for hd in range(2):
                                et = exps.tile([128, SCW], BF16, name="et", tag="et")
                                nc.scalar.activation(et[:, :qw], scs[hd][:, :qw], EXP, scale=SCALE)
                                exp_tiles[hd][k] = et
                            if fuse_own_pv:
                                for hd in range(2):
                                    for pv, (_, c0, cw, qabs) in zip(own_pvs[hd], cchunks):
                                        nc.tensor.matmul(
                                            pv[:, :cw],
                                            lhsT=vbs[chp][hd][:, k * 65:(k + 1) * 65],
                                            rhs=exp_tiles[hd][k][:, c0:c0 + cw],
                                            start=(k == 0), stop=(k == KT - 1),
                                        )
                        if prev is not None:
                            for hd in range(2):
                                for pv, (_, c0, cw, qabs) in zip(pvs_by_hd[hd], pchunks):
                                    nc.tensor.matmul(
                                        pv[:, :cw],
                                        lhsT=vb[hd][:, k * 65:(k + 1) * 65],
                                        rhs=pets[hd][k][:, c0:c0 + cw],
                                        start=(k == 0), stop=(k == KT - 1),
                                    )
                        if weave:
                            weave.pop(0)()
                    if prev is not None:
                        norm_chunks(pqh, php, [
                            (ck, pvs_by_hd[0][i], pvs_by_hd[1][i])
                            for i, ck in enumerate(pchunks)
                        ])
                    for w in weave:
                        w()
                    return exp_tiles if not fuse_own_pv else (exp_tiles, own_pvs)

                def last_phase(prev):
                    """Final PV phase, chunk-major with per-chunk normalize,
                    with that chunk's projection emitted right after."""
                    pqh, php, pets = prev
                    pchunks = [c for c in PV_CHUNKS if c[0] == pqh]
                    for ci, ck in enumerate(pchunks):
                        _, c0, cw, qabs = ck
                        pvs = []
                        for hd in range(2):
                            pv = pvp.tile([65, 512], F32, name="pv", tag="pv")
                            for k in range(KT):
                                nc.tensor.matmul(
                                    pv[:, :cw],
                                    lhsT=vbs[php][hd][:, k * 65:(k + 1) * 65],
                                    rhs=pets[hd][k][:, c0:c0 + cw],
                                    start=(k == 0), stop=(k == KT - 1),
                                )
                            pvs.append(pv)
                        norm_chunks(pqh, php, [(ck, pvs[0], pvs[1])], pe_bcast=True)
                        for mo in range(6):
                            proj_unit(mo, ck[3], ck[2])

                # software pipeline, interleaved per k tile: QK/exp of phase
                # i+1 woven with PV of phase i; a segment's projection units
                # are woven into the phase after its last PV (except the
                # final segment, handled chunk-major in last_phase).
                phases = [(qh, hp) for qh in range(2) for hp in range(HP)]
                prev = None
                pending_proj = None
                for ph in phases:
                    weave = ()
                    if pending_proj is not None:
                        weave = [lambda: None] + [
                            (lambda m=mo, c=qc: proj_unit(m, c[0], c[1]))
                            for mo in range(6)
                            for qc in PROJ_CHUNKS_QH[pending_proj]
                        ]
                        pending_proj = None
                    ets = fused_phase(ph, prev, weave)
                    if prev is not None and prev[1] == HP - 1:
                        pending_proj = prev[0]
                    prev = (ph[0], ph[1], ets)
                last_phase(prev)

            for _ in range(n_iters):
                body()
    _split_sync_waits(nc)
    return nc


def prepare_inputs(q, k, v, cls_q, cls_k, cls_v, W):
    """Host-side shard + layout prep. Returns list of 8 in_maps."""
    q = np.asarray(q, np.float32).reshape(B, C, N_SP)
    k = np.asarray(k, np.float32).reshape(B, C, N_SP)
    v = np.asarray(v, np.float32).reshape(B, C, N_SP)
    cls_q = np.asarray(cls_q, np.float32)
    cls_k = np.asarray(cls_k, np.float32)
    cls_v = np.asarray(cls_v, np.float32)
    WT = np.asarray(W, np.float32).T  # [in, out]

    in_maps = []
    for core in range(N_CORES):
        b, hf = divmod(core, 2)
        q_arr = np.zeros((HP, 128, S), np.float32)
        k_arr = np.zeros((HP, 128, SK), np.float32)
        vb_arr = np.zeros((2 * HP, 128, KT * 65), np.float32)
        wt_arr = np.zeros((HP, 128, C), np.float32)
        for hp in range(HP):
            ch0 = (hf * 6 + 2 * hp) * HD  # first channel of this head pair
            q_arr[hp, :, 0] = cls_q[b, ch0:ch0 + 128]
            q_arr[hp, :, 1:] = q[b, ch0:ch0 + 128]
            k_arr[hp, :, 0] = cls_k[b, ch0:ch0 + 128]
            k_arr[hp, :, 1:S] = k[b, ch0:ch0 + 128]
            wt_arr[hp] = WT[ch0:ch0 + 128]
            for hd in range(2):
                c0 = ch0 + hd * HD
                V = np.zeros((SK, 65), np.float32)
                V[0, :HD] = cls_v[b, c0:c0 + HD]
                V[1:S, :HD] = v[b, c0:c0 + HD].T
                V[:S, HD] = 1.0
                vb_arr[2 * hp + hd] = (
                    V.reshape(KT, 128, 65).transpose(1, 0, 2).reshape(128, KT * 65)
                )
        in_maps.append({
            "q_in": q_arr.astype(_bf16),
            "k_in": k_arr.astype(_bf16),
            "vb_in": vb_arr.astype(_bf16),
            "wt_in": wt_arr.astype(_bf16),
        })
    return in_maps


def assemble_output(results, bias):
    bias = np.asarray(bias, np.float32)
    x = np.empty((B, C, L, H, W_SP), np.float32)
    cls_tok = np.empty((B, C), np.float32)
    for b in range(B):
        yT = results[2 * b]["y_out"].reshape(C, S) + results[2 * b + 1]["y_out"].reshape(C, S)
        yT += bias[:, None]
        cls_tok[b] = yT[:, 0]
        x[b] = yT[:, 1:].reshape(C, L, H, W_SP)
    return x, cls_tok


_program_cache = {}


def _get_program(n_iters):
    if n_iters not in _program_cache:
        _program_cache[n_iters] = build_program(n_iters)
    return _program_cache[n_iters]


def kernel(q, k, v, cls_q, cls_k, cls_v, W, b, _n_iters=1):
    nc = _get_program(_n_iters)
    in_maps = prepare_inputs(q, k, v, cls_q, cls_k, cls_v, W)
    res = run_bass_kernel_spmd(nc, in_maps, core_ids=list(range(N_CORES)))
    return assemble_output(res.results, b)
